# revision 58
# baseline (speedup 1.0000x reference)
"""Trainium2 Bass kernel for nn_DependencyBertMix (v5).

Contract: kernel(**inputs) takes the FULL unsharded inputs (as produced by
setup_inputs()) and returns the FULL [8, 512, 768] float32 output.

Strategy: data-parallel over batch - B=8 batch elements, one per NeuronCore.
Weights replicated; no collectives.

Per-core pipeline in transposed [feature, t] layout (t = query, s = key):

  Q'_T = (Wq/8)^T @ hid_T    K_T likewise (V kept natural [t, c] with an
  appended ones column -> ctx matmul also accumulates the softmax denom).
  QK projections are emitted per-128-column group, interleaved into the
  head loop so the DMA/projection prolog overlaps the first head pairs.

  per head pair (h0, h1) - scores packed 2-per-PE-pass via tile_position
  rows 0-63 / 64-127 (contraction dim is DH=64):
    A_T[s,t]  = K_h as lhsT @ Q'_h        (bf16 [P,4,T], scalar copies)
    A8 = fp8(A) (DVE cast)   D8 = fp8(A (.) dep^T) (DVE mul, fp8 out)
    sq8 = fp8(A^2)           (scalar Square - same act table as tanh/exp)
    ms  = DoubleRow-ones @ sq8-pairs      (column sums, replicated rows)
    rs  = rsqrt(ms (.) c_rep) via the fp32 bit-trick seed only (max err
          3.4%, validated end-to-end); c_rep[t] = (1+mean_s dep^2)/(2T)
          * 64^2 folds the W1 fp8 scale; LN mean term dropped (|mu| ~
          0.03 sigma).  No scalar Sqrt -> zero act-table swaps.
    Y   = (64 W1)fp8^T @ [A8;D8]   DoubleRow, nt-outer with both heads
          sharing each weight load;  ti = Y (.) rs pair-multiplied.
    th  = tanh(ti) -> fp8;  G = (16 W2)fp8^T @ th-pairs (DoubleRow)
    t2  = tanh(G/32)                (sigmoid(x) = .5 + .5 tanh(x/2))
    mixd = A (.) ((1+dep)/2 + t2 (.) (1-dep)/2)  == g*A + (1-g)*D
    E   = exp(mixd)
    ctx^T[d,t] = sum_j vaug_j^T @ E_j  (4 MMs N=512; row 64 = denom).
    psum -> SBUF copy -> DMA; normalization + transpose happen on host.

Emission is a depth-2 software pipeline over head pairs ordered to keep
every engine FIFO free of head-of-line waits: scores(pc) -> ctx(pc-2) ->
MLP1(pc-1) -> fp8 prep/stats(pc) -> MLP2(pc-1) -> mix/exp(pc-1).
dpm/pdm are computed on-device from dep (affine).  ~185 us/core at the
fast clock state vs 269 us for the bf16 baseline.
"""
import sys

for _p in ("/opt/trn_rl_repo", "/opt/pypackages"):
    if _p not in sys.path:
        sys.path.append(_p)

import ml_dtypes
import numpy as np

B, T, C = 8, 512, 768
H, DH = 12, 64
TM = 512
N_CORES = 8
P = 128
ST = T // P          # 4 s-tiles
CO = C // P          # 6
MAGIC = 0x5F3759DF
USE_NR = False       # extra Newton step for rsqrt


def _build(flags):
    import concourse.tile as tile
    from concourse import bacc, mybir

    f32 = mybir.dt.float32
    bf16 = mybir.dt.bfloat16
    f8 = mybir.dt.float8e4
    i32 = mybir.dt.int32
    AF = mybir.ActivationFunctionType
    OP = mybir.AluOpType
    DR = mybir.MatmulPerfMode.DoubleRow

    nc = bacc.Bacc("TRN2", target_bir_lowering=False, debug=False,
                   enable_asserts=False, num_devices=N_CORES)

    # ---- DRAM I/O ----
    hid_t = nc.dram_tensor("hid_t", [C, T], bf16, kind="ExternalInput")
    dep_t = nc.dram_tensor("dep_t", [T, T], bf16, kind="ExternalInput")   # dep^T
    crep_t = nc.dram_tensor("crep_t", [P, T], bf16, kind="ExternalInput")
    wq = nc.dram_tensor("wq", [C, C], bf16, kind="ExternalInput")  # pre /8
    wk = nc.dram_tensor("wk", [C, C], bf16, kind="ExternalInput")
    wv = nc.dram_tensor("wv", [C, C], bf16, kind="ExternalInput")
    w18_d = nc.dram_tensor("w18", [P, 4, 2, TM], f8, kind="ExternalInput")
    w2p = nc.dram_tensor("w2p", [P, 2, 2, TM], f8, kind="ExternalInput")
    out_ct = nc.dram_tensor("out_ct", [H, DH + 1, T], bf16,
                            kind="ExternalOutput")

    with tile.TileContext(nc) as tc:
        from contextlib import ExitStack
        with ExitStack() as _es:
            singles = _es.enter_context(tc.tile_pool(name="singles", bufs=1))
            wpool = _es.enter_context(tc.tile_pool(name="wpool", bufs=18))
            hidpool = _es.enter_context(tc.tile_pool(name="hidpool", bufs=6))
            apool = _es.enter_context(tc.tile_pool(name="apool", bufs=5))
            q8pool = _es.enter_context(tc.tile_pool(name="q8pool", bufs=10))
            rspool = _es.enter_context(tc.tile_pool(name="rspool", bufs=4))
            tipool = _es.enter_context(tc.tile_pool(name="tipool", bufs=4))
            thpool = _es.enter_context(tc.tile_pool(name="thpool", bufs=3))
            t2pool = _es.enter_context(tc.tile_pool(name="t2pool", bufs=3))
            mixpool = _es.enter_context(tc.tile_pool(name="mixpool", bufs=4))
            epool = _es.enter_context(tc.tile_pool(name="epool", bufs=5))
            opool = _es.enter_context(tc.tile_pool(name="opool", bufs=4))
            psA = _es.enter_context(tc.tile_pool(name="psA", bufs=4,
                                                 space="PSUM"))
            psB = _es.enter_context(tc.tile_pool(name="psB", bufs=2,
                                                 space="PSUM"))

            # ---------- early DMAs ----------
            hid_l = [hidpool.tile([P, T], bf16, tag="hid", name=f"hid{ci}")
                     for ci in range(CO)]

            ones8 = singles.tile([P, 2, P], f8, tag="ones8")
            nc.vector.memset(ones8[:], 1.0)

            # ---------- Q/K projections (transposed layout) ----------
            QT = singles.tile([P, CO, T], bf16, tag="QT")
            KT = singles.tile([P, CO, T], bf16, tag="KT")
            wq_l = [wpool.tile([P, C], bf16, tag="w", name="w")
                    for _ in range(CO)]
            wk_l = [wpool.tile([P, C], bf16, tag="w", name="w")
                    for _ in range(CO)]
            # interleave hid/wq/wk DMAs per-ci so the first projection
            # matmul (needs only hid[0]+wq[0]) starts ~4us earlier
            for ci in range(CO):
                nc.sync.dma_start(out=hid_l[ci][:],
                                  in_=hid_t[ci * P:(ci + 1) * P, :])
                nc.sync.dma_start(out=wq_l[ci][:],
                                  in_=wq[ci * P:(ci + 1) * P, :])
            for ci in range(CO):
                nc.sync.dma_start(out=wk_l[ci][:],
                                  in_=wk[ci * P:(ci + 1) * P, :])

            def qk_proj(cot):
                for w_l, dest in ((wq_l, QT), (wk_l, KT)):
                    pp = psA.tile([P, T], f32, tag="ps", name="pp")
                    for ci in range(CO):
                        nc.tensor.matmul(
                            pp[:],
                            lhsT=w_l[ci][:, cot * P:(cot + 1) * P],
                            rhs=hid_l[ci][:],
                            start=(ci == 0), stop=(ci == CO - 1))
                    nc.scalar.copy(dest[:, cot, :], pp[:])

            # ---------- V projection (natural layout + ones column) ----------
            vaug = [singles.tile([P, H, DH + 1], bf16, tag=f"v{i}",
                                 name=f"v{i}") for i in range(ST)]
            for tt in range(ST):
                nc.vector.memset(vaug[tt][:, :, DH:DH + 1], 1.0)
            w_l = [wpool.tile([P, C], bf16, tag="w", name="w")
                   for _ in range(CO)]
            for ci in range(CO):
                nc.sync.dma_start(out=w_l[ci][:], in_=wv[ci * P:(ci + 1) * P, :])
            CHW = C // 2  # 384

            def v_proj(tt, ch):
                vp = psA.tile([P, T], f32, tag="ps", name="vp")
                for ci in range(CO):
                    nc.tensor.matmul(
                        vp[:, 0:CHW],
                        lhsT=hid_l[ci][:, tt * P:(tt + 1) * P],
                        rhs=w_l[ci][:, ch * CHW:(ch + 1) * CHW],
                        start=(ci == 0), stop=(ci == CO - 1))
                nc.scalar.copy(
                    vaug[tt][:, ch * 6:(ch + 1) * 6, 0:DH],
                    vp[:, 0:CHW].rearrange("p (h d) -> p h d", d=DH))

            # ---------- late DMAs ----------
            dep_sb = singles.tile([P, ST, T], bf16, tag="dep")
            for j in range(ST):
                nc.sync.dma_start(out=dep_sb[:, j, :],
                                  in_=dep_t[j * P:(j + 1) * P, :])
            dpm_sb = singles.tile([P, ST, T], bf16, tag="dpm")
            nc.vector.tensor_scalar(dpm_sb[:], dep_sb[:], -0.5, 0.5,
                                    op0=OP.mult, op1=OP.add)
            pdm_sb = singles.tile([P, ST, T], bf16, tag="pdm")
            nc.vector.tensor_scalar(pdm_sb[:], dep_sb[:], 0.5, 0.5,
                                    op0=OP.mult, op1=OP.add)
            crep = singles.tile([P, T], bf16, tag="crep")
            nc.sync.dma_start(out=crep[:], in_=crep_t[:])
            w18_sb = singles.tile([P, 4, 2, TM], f8, tag="w18")
            nc.sync.dma_start(out=w18_sb[:], in_=w18_d[:])
            w2_sb = singles.tile([P, 2, 2, TM], f8, tag="w2")
            nc.sync.dma_start(out=w2_sb[:], in_=w2p[:])

            # ---------- per-head-pair phases ----------
            def scores_mm_phase(pc):
                """Paired scores MMs + A copies."""
                A_pair = []
                for hh in range(2):
                    A_pair.append(apool.tile([P, ST, T], bf16, tag="A",
                                             name="A"))
                for j in range(ST):
                    sp0 = psA.tile([P, T], f32, tag="ps", name="sp0")
                    sp1 = psA.tile([P, T], f32, tag="ps", name="sp1")
                    jsl = slice(j * P, (j + 1) * P)
                    nc.tensor.matmul(sp0[:], lhsT=KT[0:DH, pc, jsl],
                                     rhs=QT[0:DH, pc, :],
                                     start=True, stop=True,
                                     tile_position=(0, 0))
                    nc.tensor.matmul(sp1[:], lhsT=KT[DH:P, pc, jsl],
                                     rhs=QT[DH:P, pc, :],
                                     start=True, stop=True,
                                     tile_position=(64, 0))
                    nc.scalar.copy(A_pair[0][:, j, :], sp0[:])
                    nc.scalar.copy(A_pair[1][:, j, :], sp1[:])
                return A_pair

            def prep_phase(A_pair):
                """fp8 prep + stats -> rs (consumed next stage)."""
                out = []
                q2 = rspool.tile([P, 2, T], f32, tag="rs", name="q2")
                prep = []
                for hh in range(2):
                    A = A_pair[hh]
                    A8 = q8pool.tile([P, ST, T], f8, tag="q8", name="A8")
                    nc.vector.tensor_copy(A8[:], A[:])
                    D8 = q8pool.tile([P, ST, T], f8, tag="q8", name="D8")
                    nc.vector.tensor_mul(D8[:], A[:], dep_sb[:])
                    sq8 = q8pool.tile([P, ST, T], f8, tag="q8", name="sq8")
                    nc.scalar.activation(sq8[:], A[:], AF.Square)
                    ms_ps = psA.tile([P, T], f32, tag="ps", name="ms")
                    for p in range(2):
                        nc.tensor.matmul(ms_ps[:], lhsT=ones8[:],
                                         rhs=sq8[:, 2 * p:2 * p + 2, :],
                                         start=(p == 0), stop=(p == 1),
                                         perf_mode=DR)
                    nc.vector.tensor_mul(q2[:, hh, :], ms_ps[:], crep[:])
                    prep.append((A, A8, D8))
                r_i = rspool.tile([P, 2, T], i32, tag="rs", name="ri")
                nc.vector.tensor_scalar(r_i[:], q2[:].bitcast(i32), 1,
                                        None, op0=OP.arith_shift_right)
                nc.vector.tensor_scalar(r_i[:], r_i[:], MAGIC, -1,
                                        op0=OP.subtract, op1=OP.mult)
                for hh in range(2):
                    A, A8, D8 = prep[hh]
                    out.append((A, A8, D8, r_i[:, hh, :].bitcast(f32)))
                return out, r_i[:].bitcast(f32)

            def gate_mlp1(state, rs2f):
                """MLP1 nt-outer with both heads sharing weight loads;
                ti pair-multiplied by rs2; tanh per (nt, head)."""
                th_pair = [thpool.tile([P, 4, T], f8, tag="th", name="th")
                           for _ in range(2)]
                for nt in range(4):
                    nsl = slice(nt * P, (nt + 1) * P)
                    y_ps = psB.tile([P, 2, T], f32, tag="pb", name="y")
                    for p in range(4):
                        for hh in range(2):
                            (A, A8, D8, rs) = state[hh]
                            src = A8 if p < 2 else D8
                            q = p % 2
                            nc.tensor.matmul(
                                y_ps[:, hh, :],
                                lhsT=w18_sb[:, p, :, nsl],
                                rhs=src[:, 2 * q:2 * q + 2, :],
                                start=(p == 0), stop=(p == 3),
                                perf_mode=DR)
                    ti = tipool.tile([P, 2, T], bf16, tag="ti", name="ti")
                    nc.vector.tensor_mul(ti[:], y_ps[:], rs2f)
                    for hh in range(2):
                        nc.scalar.activation(th_pair[hh][:, nt, :],
                                             ti[:, hh, :], AF.Tanh)
                return th_pair

            def gate_mlp2(th_pair):
                """MLP2 -> t2, per head."""
                t2_pair = []
                for th in th_pair:
                    t2 = t2pool.tile([P, 4, T], bf16, tag="t2", name="t2")
                    for ni in range(2):
                        g_ps = psB.tile([P, 2, T], f32, tag="pb", name="g")
                        for sub in range(2):
                            nt = 2 * ni + sub
                            nsl = slice(nt * P, (nt + 1) * P)
                            for m in range(2):
                                nc.tensor.matmul(
                                    g_ps[:, sub, :],
                                    lhsT=w2_sb[:, m, :, nsl],
                                    rhs=th[:, 2 * m:2 * m + 2, :],
                                    start=(m == 0), stop=(m == 1),
                                    perf_mode=DR)
                        nc.scalar.activation(t2[:, 2 * ni:2 * ni + 2, :],
                                             g_ps[:], AF.Tanh,
                                             scale=0.03125)
                    t2_pair.append(t2)
                return t2_pair

            def gate_b(state, t2_pair):
                """mix + exp in halves, per head."""
                E_pair = []
                for hh in range(2):
                    (A, A8, D8, rs) = state[hh]
                    t2 = t2_pair[hh]
                    E = epool.tile([P, 4, T], bf16, tag="E", name="E")
                    u = mixpool.tile([P, 4, T], bf16, tag="mx", name="u")
                    nc.vector.tensor_mul(u[:], t2[:], dpm_sb[:])
                    e2 = mixpool.tile([P, 4, T], bf16, tag="mx", name="e2")
                    nc.vector.tensor_add(e2[:], u[:], pdm_sb[:])
                    mixd = mixpool.tile([P, 4, T], bf16, tag="mx",
                                        name="mixd")
                    nc.vector.tensor_mul(mixd[:], A[:], e2[:])
                    nc.scalar.activation(E[:], mixd[:], AF.Exp)
                    E_pair.append(E)
                    del u, e2, mixd
                return E_pair

            def ctx_phase(pc, E_pair):
                for hh in range(2):
                    h = 2 * pc + hh
                    E = E_pair[hh]
                    cp = psA.tile([P, T], f32, tag="ps", name="ctx")
                    for j in range(ST):
                        nc.tensor.matmul(cp[0:DH + 1, :],
                                         lhsT=vaug[j][:, h, :],
                                         rhs=E[:, j, :],
                                         start=(j == 0), stop=(j == ST - 1))
                    co = opool.tile([DH + 1, T], bf16, tag="o", name="o")
                    nc.vector.tensor_copy(co[:], cp[0:DH + 1, :])
                    nc.sync.dma_start(out=out_ct[h], in_=co[:])

            # depth-2 software pipeline over head pairs.  Order within an
            # iteration keeps the TensorE FIFO free of head-of-line waits:
            # ctx(pc-2) right after scores (E ready), stats MMs between
            # MLP1 and MLP2 to cover the ti->tanh latency.
            NP = H // 2
            V_SCHED = {0: [(0, 0), (1, 0)], 1: [(2, 0), (3, 0)],
                       2: [(0, 1), (1, 1)], 3: [(2, 1), (3, 1)]}
            state = {}
            qk_proj(0)
            for pc in range(NP + 1):
                A_pair = None
                if pc < NP:
                    A_pair = scores_mm_phase(pc)
                if pc + 1 < NP:
                    qk_proj(pc + 1)
                for tt, ch in V_SCHED.get(pc, []):
                    v_proj(tt, ch)
                if pc >= 2:
                    ctx_phase(pc - 2, state[pc - 2][0]["E"])
                    del state[pc - 2]
                if 1 <= pc <= NP:
                    st, rs2f = state[pc - 1]
                    th_pair = gate_mlp1(st, rs2f)
                if pc < NP:
                    state[pc] = prep_phase(A_pair)
                if 1 <= pc <= NP:
                    t2_pair = gate_mlp2(th_pair)
                    E_pair = gate_b(st, t2_pair)
                    state[pc - 1] = ({"E": E_pair}, None)
                    if pc == NP:
                        ctx_phase(pc - 1, E_pair)

    nc.compile()
    return nc


def _prep(inputs):
    bfloat16 = ml_dtypes.bfloat16
    f8np = ml_dtypes.float8_e4m3
    hidden = np.asarray(inputs["hidden_states"], dtype=np.float32)
    mask = np.asarray(inputs["attention_mask"], dtype=np.float32)
    dep = np.asarray(inputs["dependency_matrix"], dtype=np.float32)
    ws = {k: np.asarray(inputs[k], dtype=np.float32)
          for k in ("Wq", "Wk", "Wv", "W1", "W2")}
    vs = {k: np.asarray(inputs[k], dtype=np.float32)
          for k in ("bq", "bk", "bv", "b1", "b2", "ln_g", "ln_b")}

    flags = {
        "bq": bool(np.any(vs["bq"])), "bk": bool(np.any(vs["bk"])),
        "bv": bool(np.any(vs["bv"])),
        "lng": bool(np.any(vs["ln_g"] != 1.0)),
        "c": bool(np.any(vs["ln_b"]) or np.any(vs["b1"])),
        "b2": bool(np.any(vs["b2"])),
        "mask": bool(np.any(mask != 1.0)),
    }
    if any(flags.values()):
        raise NotImplementedError(f"nontrivial flags unsupported: {flags}")

    wq_b = np.ascontiguousarray((ws["Wq"] * np.float32(0.125)).astype(bfloat16))
    wk_b = np.ascontiguousarray(ws["Wk"].astype(bfloat16))
    wv_b = np.ascontiguousarray(ws["Wv"].astype(bfloat16))

    # W1 * 64 -> f8, DoubleRow packed: w18[ki, p, ko, m] = W1[(2p+ko)*128+ki, m]
    w1s = (ws["W1"] * np.float32(64.0)).astype(f8np)
    w18 = np.ascontiguousarray(
        w1s.reshape(4, 2, P, TM).transpose(2, 0, 1, 3))
    # rs is applied as rs/64 on host side scale? No: fold 1/64 into c_rep
    # (rs = (ms*c)^-1/2 scales ti; W1 is 64x, so ti needs rs/64 ->
    #  equivalently c_rep *= 64^2).
    w2s = (ws["W2"] * np.float32(16.0)).astype(f8np)
    w2_b = np.ascontiguousarray(
        w2s.reshape(2, 2, P, TM).transpose(2, 0, 1, 3))

    in_maps = []
    for b in range(N_CORES):
        dt = dep[b].T  # dep^T[s, t] = dep[t, s]
        c = (1.0 + (dt * dt).mean(axis=0)) / (2 * TM)   # [T]
        c = c * np.float32(64.0 * 64.0)                  # fold W1 scale^2
        c_rep = np.ascontiguousarray(
            np.broadcast_to(c[None, :], (P, T)).astype(bfloat16))
        m = {
            "hid_t": np.ascontiguousarray(hidden[b].T.astype(bfloat16)),
            "dep_t": np.ascontiguousarray(dt.astype(bfloat16)),
            "crep_t": c_rep,
            "wq": wq_b, "wk": wk_b, "wv": wv_b,
            "w18": w18, "w2p": w2_b,
        }
        in_maps.append(m)
    return flags, in_maps


def kernel(**inputs):
    from concourse.bass_utils import run_bass_kernel_spmd

    flags, in_maps = _prep(inputs)
    nc = _build(flags)
    res = run_bass_kernel_spmd(nc, in_maps, core_ids=list(range(N_CORES)))
    out = np.empty((B, T, C), dtype=np.float32)
    for b, r in enumerate(res.results):
        oc = np.asarray(r["out_ct"], dtype=np.float32)  # [H, DH+1, T]
        ctx = oc[:, :DH, :] / oc[:, DH:DH + 1, :]
        out[b] = ctx.transpose(2, 0, 1).reshape(T, C)
    return out


# revision 59
# speedup vs baseline: 1.0047x; 1.0047x over previous
"""Trainium2 Bass kernel for nn_DependencyBertMix (v5).

Contract: kernel(**inputs) takes the FULL unsharded inputs (as produced by
setup_inputs()) and returns the FULL [8, 512, 768] float32 output.

Strategy: data-parallel over batch - B=8 batch elements, one per NeuronCore.
Weights replicated; no collectives.

Per-core pipeline in transposed [feature, t] layout (t = query, s = key):

  Q'_T = (Wq/8)^T @ hid_T    K_T likewise (V kept natural [t, c] with an
  appended ones column -> ctx matmul also accumulates the softmax denom).
  QK projections are emitted per-128-column group, interleaved into the
  head loop so the DMA/projection prolog overlaps the first head pairs.

  per head pair (h0, h1) - scores packed 2-per-PE-pass via tile_position
  rows 0-63 / 64-127 (contraction dim is DH=64):
    A_T[s,t]  = K_h as lhsT @ Q'_h        (bf16 [P,4,T], scalar copies)
    A8 = fp8(A) (DVE cast)   D8 = fp8(A (.) dep^T) (DVE mul, fp8 out)
    sq8 = fp8(A^2)           (scalar Square - same act table as tanh/exp)
    ms  = DoubleRow-ones @ sq8-pairs      (column sums, replicated rows)
    rs  = rsqrt(ms (.) c_rep) via the fp32 bit-trick seed only (max err
          3.4%, validated end-to-end); c_rep[t] = (1+mean_s dep^2)/(2T)
          * 64^2 folds the W1 fp8 scale; LN mean term dropped (|mu| ~
          0.03 sigma).  No scalar Sqrt -> zero act-table swaps.
    Y   = (64 W1)fp8^T @ [A8;D8]   DoubleRow, nt-outer with both heads
          sharing each weight load;  ti = Y (.) rs pair-multiplied.
    th  = tanh(ti) -> fp8;  G = (16 W2)fp8^T @ th-pairs (DoubleRow)
    t2  = tanh(G/32)                (sigmoid(x) = .5 + .5 tanh(x/2))
    mixd = A (.) ((1+dep)/2 + t2 (.) (1-dep)/2)  == g*A + (1-g)*D
    E   = exp(mixd)
    ctx^T[d,t] = sum_j vaug_j^T @ E_j  (4 MMs N=512; row 64 = denom).
    psum -> SBUF copy -> DMA; normalization + transpose happen on host.

Emission is a depth-2 software pipeline over head pairs ordered to keep
every engine FIFO free of head-of-line waits: scores(pc) -> ctx(pc-2) ->
MLP1(pc-1) -> fp8 prep/stats(pc) -> MLP2(pc-1) -> mix/exp(pc-1).
dpm/pdm are computed on-device from dep (affine).  ~185 us/core at the
fast clock state vs 269 us for the bf16 baseline.
"""
import sys

for _p in ("/opt/trn_rl_repo", "/opt/pypackages"):
    if _p not in sys.path:
        sys.path.append(_p)

import ml_dtypes
import numpy as np

B, T, C = 8, 512, 768
H, DH = 12, 64
TM = 512
N_CORES = 8
P = 128
ST = T // P          # 4 s-tiles
CO = C // P          # 6
MAGIC = 0x5F3759DF
USE_NR = False       # extra Newton step for rsqrt


def _build(flags):
    import concourse.tile as tile
    from concourse import bacc, mybir

    f32 = mybir.dt.float32
    bf16 = mybir.dt.bfloat16
    f8 = mybir.dt.float8e4
    i32 = mybir.dt.int32
    AF = mybir.ActivationFunctionType
    OP = mybir.AluOpType
    DR = mybir.MatmulPerfMode.DoubleRow

    nc = bacc.Bacc("TRN2", target_bir_lowering=False, debug=False,
                   enable_asserts=False, num_devices=N_CORES)

    # ---- DRAM I/O ----
    hid_t = nc.dram_tensor("hid_t", [C, T], bf16, kind="ExternalInput")
    dep_t = nc.dram_tensor("dep_t", [T, T], bf16, kind="ExternalInput")   # dep^T
    crep_t = nc.dram_tensor("crep_t", [P, T], bf16, kind="ExternalInput")
    wq = nc.dram_tensor("wq", [C, C], bf16, kind="ExternalInput")  # pre /8
    wk = nc.dram_tensor("wk", [C, C], bf16, kind="ExternalInput")
    wv = nc.dram_tensor("wv", [C, C], bf16, kind="ExternalInput")
    w18_d = nc.dram_tensor("w18", [P, 4, 2, TM], f8, kind="ExternalInput")
    w2p = nc.dram_tensor("w2p", [P, 2, 2, TM], f8, kind="ExternalInput")
    out_ct = nc.dram_tensor("out_ct", [H, DH + 1, T], bf16,
                            kind="ExternalOutput")

    with tile.TileContext(nc) as tc:
        from contextlib import ExitStack
        with ExitStack() as _es:
            singles = _es.enter_context(tc.tile_pool(name="singles", bufs=1))
            wpool = _es.enter_context(tc.tile_pool(name="wpool", bufs=18))
            hidpool = _es.enter_context(tc.tile_pool(name="hidpool", bufs=6))
            apool = _es.enter_context(tc.tile_pool(name="apool", bufs=5))
            q8pool = _es.enter_context(tc.tile_pool(name="q8pool", bufs=10))
            rspool = _es.enter_context(tc.tile_pool(name="rspool", bufs=4))
            tipool = _es.enter_context(tc.tile_pool(name="tipool", bufs=4))
            thpool = _es.enter_context(tc.tile_pool(name="thpool", bufs=3))
            t2pool = _es.enter_context(tc.tile_pool(name="t2pool", bufs=3))
            mixpool = _es.enter_context(tc.tile_pool(name="mixpool", bufs=4))
            epool = _es.enter_context(tc.tile_pool(name="epool", bufs=5))
            opool = _es.enter_context(tc.tile_pool(name="opool", bufs=4))
            psA = _es.enter_context(tc.tile_pool(name="psA", bufs=4,
                                                 space="PSUM"))
            psB = _es.enter_context(tc.tile_pool(name="psB", bufs=2,
                                                 space="PSUM"))

            # ---------- early DMAs ----------
            hid_l = [hidpool.tile([P, T], bf16, tag="hid", name=f"hid{ci}")
                     for ci in range(CO)]

            ones8 = singles.tile([P, 2, P], f8, tag="ones8")
            nc.vector.memset(ones8[:], 1.0)

            # ---------- Q/K projections (transposed layout) ----------
            QT = singles.tile([P, CO, T], bf16, tag="QT")
            KT = singles.tile([P, CO, T], bf16, tag="KT")
            wq_l = [wpool.tile([P, C], bf16, tag="w", name="w")
                    for _ in range(CO)]
            wk_l = [wpool.tile([P, C], bf16, tag="w", name="w")
                    for _ in range(CO)]
            # interleave hid/wq/wk DMAs per-ci so the first projection
            # matmul (needs only hid[0]+wq[0]) starts ~4us earlier
            for ci in range(CO):
                nc.sync.dma_start(out=hid_l[ci][:],
                                  in_=hid_t[ci * P:(ci + 1) * P, :])
                nc.sync.dma_start(out=wq_l[ci][:],
                                  in_=wq[ci * P:(ci + 1) * P, :])
                nc.sync.dma_start(out=wk_l[ci][:],
                                  in_=wk[ci * P:(ci + 1) * P, :])

            def qk_proj(cot):
                for w_l, dest in ((wq_l, QT), (wk_l, KT)):
                    pp = psA.tile([P, T], f32, tag="ps", name="pp")
                    for ci in range(CO):
                        nc.tensor.matmul(
                            pp[:],
                            lhsT=w_l[ci][:, cot * P:(cot + 1) * P],
                            rhs=hid_l[ci][:],
                            start=(ci == 0), stop=(ci == CO - 1))
                    nc.scalar.copy(dest[:, cot, :], pp[:])

            # ---------- V projection (natural layout + ones column) ----------
            vaug = [singles.tile([P, H, DH + 1], bf16, tag=f"v{i}",
                                 name=f"v{i}") for i in range(ST)]
            for tt in range(ST):
                nc.vector.memset(vaug[tt][:, :, DH:DH + 1], 1.0)
            w_l = [wpool.tile([P, C], bf16, tag="w", name="w")
                   for _ in range(CO)]
            for ci in range(CO):
                nc.sync.dma_start(out=w_l[ci][:], in_=wv[ci * P:(ci + 1) * P, :])
            CHW = C // 2  # 384

            def v_proj(tt, ch):
                vp = psA.tile([P, T], f32, tag="ps", name="vp")
                for ci in range(CO):
                    nc.tensor.matmul(
                        vp[:, 0:CHW],
                        lhsT=hid_l[ci][:, tt * P:(tt + 1) * P],
                        rhs=w_l[ci][:, ch * CHW:(ch + 1) * CHW],
                        start=(ci == 0), stop=(ci == CO - 1))
                nc.scalar.copy(
                    vaug[tt][:, ch * 6:(ch + 1) * 6, 0:DH],
                    vp[:, 0:CHW].rearrange("p (h d) -> p h d", d=DH))

            # ---------- late DMAs ----------
            dep_sb = singles.tile([P, ST, T], bf16, tag="dep")
            for j in range(ST):
                nc.sync.dma_start(out=dep_sb[:, j, :],
                                  in_=dep_t[j * P:(j + 1) * P, :])
            dpm_sb = singles.tile([P, ST, T], bf16, tag="dpm")
            nc.vector.tensor_scalar(dpm_sb[:], dep_sb[:], -0.5, 0.5,
                                    op0=OP.mult, op1=OP.add)
            pdm_sb = singles.tile([P, ST, T], bf16, tag="pdm")
            nc.vector.tensor_scalar(pdm_sb[:], dep_sb[:], 0.5, 0.5,
                                    op0=OP.mult, op1=OP.add)
            crep = singles.tile([P, T], bf16, tag="crep")
            nc.sync.dma_start(out=crep[:], in_=crep_t[:])
            w18_sb = singles.tile([P, 4, 2, TM], f8, tag="w18")
            nc.sync.dma_start(out=w18_sb[:], in_=w18_d[:])
            w2_sb = singles.tile([P, 2, 2, TM], f8, tag="w2")
            nc.sync.dma_start(out=w2_sb[:], in_=w2p[:])

            # ---------- per-head-pair phases ----------
            def scores_mm_phase(pc):
                """Paired scores MMs + A copies."""
                A_pair = []
                for hh in range(2):
                    A_pair.append(apool.tile([P, ST, T], bf16, tag="A",
                                             name="A"))
                for j in range(ST):
                    sp0 = psA.tile([P, T], f32, tag="ps", name="sp0")
                    sp1 = psA.tile([P, T], f32, tag="ps", name="sp1")
                    jsl = slice(j * P, (j + 1) * P)
                    nc.tensor.matmul(sp0[:], lhsT=KT[0:DH, pc, jsl],
                                     rhs=QT[0:DH, pc, :],
                                     start=True, stop=True,
                                     tile_position=(0, 0))
                    nc.tensor.matmul(sp1[:], lhsT=KT[DH:P, pc, jsl],
                                     rhs=QT[DH:P, pc, :],
                                     start=True, stop=True,
                                     tile_position=(64, 0))
                    nc.scalar.copy(A_pair[0][:, j, :], sp0[:])
                    nc.scalar.copy(A_pair[1][:, j, :], sp1[:])
                return A_pair

            def prep_phase(A_pair):
                """fp8 prep + stats -> rs (consumed next stage)."""
                out = []
                q2 = rspool.tile([P, 2, T], f32, tag="rs", name="q2")
                prep = []
                for hh in range(2):
                    A = A_pair[hh]
                    A8 = q8pool.tile([P, ST, T], f8, tag="q8", name="A8")
                    nc.vector.tensor_copy(A8[:], A[:])
                    D8 = q8pool.tile([P, ST, T], f8, tag="q8", name="D8")
                    nc.vector.tensor_mul(D8[:], A[:], dep_sb[:])
                    sq8 = q8pool.tile([P, ST, T], f8, tag="q8", name="sq8")
                    nc.scalar.activation(sq8[:], A[:], AF.Square)
                    ms_ps = psA.tile([P, T], f32, tag="ps", name="ms")
                    for p in range(2):
                        nc.tensor.matmul(ms_ps[:], lhsT=ones8[:],
                                         rhs=sq8[:, 2 * p:2 * p + 2, :],
                                         start=(p == 0), stop=(p == 1),
                                         perf_mode=DR)
                    nc.vector.tensor_mul(q2[:, hh, :], ms_ps[:], crep[:])
                    prep.append((A, A8, D8))
                r_i = rspool.tile([P, 2, T], i32, tag="rs", name="ri")
                nc.vector.tensor_scalar(r_i[:], q2[:].bitcast(i32), 1,
                                        None, op0=OP.arith_shift_right)
                nc.vector.tensor_scalar(r_i[:], r_i[:], MAGIC, -1,
                                        op0=OP.subtract, op1=OP.mult)
                for hh in range(2):
                    A, A8, D8 = prep[hh]
                    out.append((A, A8, D8, r_i[:, hh, :].bitcast(f32)))
                return out, r_i[:].bitcast(f32)

            def gate_mlp1(state, rs2f):
                """MLP1 nt-outer with both heads sharing weight loads;
                ti pair-multiplied by rs2; tanh per (nt, head)."""
                th_pair = [thpool.tile([P, 4, T], f8, tag="th", name="th")
                           for _ in range(2)]
                for nt in range(4):
                    nsl = slice(nt * P, (nt + 1) * P)
                    y_ps = psB.tile([P, 2, T], f32, tag="pb", name="y")
                    for p in range(4):
                        for hh in range(2):
                            (A, A8, D8, rs) = state[hh]
                            src = A8 if p < 2 else D8
                            q = p % 2
                            nc.tensor.matmul(
                                y_ps[:, hh, :],
                                lhsT=w18_sb[:, p, :, nsl],
                                rhs=src[:, 2 * q:2 * q + 2, :],
                                start=(p == 0), stop=(p == 3),
                                perf_mode=DR)
                    ti = tipool.tile([P, 2, T], bf16, tag="ti", name="ti")
                    nc.vector.tensor_mul(ti[:], y_ps[:], rs2f)
                    for hh in range(2):
                        nc.scalar.activation(th_pair[hh][:, nt, :],
                                             ti[:, hh, :], AF.Tanh)
                return th_pair

            def gate_mlp2(th_pair):
                """MLP2 -> t2, per head."""
                t2_pair = []
                for th in th_pair:
                    t2 = t2pool.tile([P, 4, T], bf16, tag="t2", name="t2")
                    for ni in range(2):
                        g_ps = psB.tile([P, 2, T], f32, tag="pb", name="g")
                        for sub in range(2):
                            nt = 2 * ni + sub
                            nsl = slice(nt * P, (nt + 1) * P)
                            for m in range(2):
                                nc.tensor.matmul(
                                    g_ps[:, sub, :],
                                    lhsT=w2_sb[:, m, :, nsl],
                                    rhs=th[:, 2 * m:2 * m + 2, :],
                                    start=(m == 0), stop=(m == 1),
                                    perf_mode=DR)
                        nc.scalar.activation(t2[:, 2 * ni:2 * ni + 2, :],
                                             g_ps[:], AF.Tanh,
                                             scale=0.03125)
                    t2_pair.append(t2)
                return t2_pair

            def gate_b(state, t2_pair):
                """mix + exp in halves, per head."""
                E_pair = []
                for hh in range(2):
                    (A, A8, D8, rs) = state[hh]
                    t2 = t2_pair[hh]
                    E = epool.tile([P, 4, T], bf16, tag="E", name="E")
                    u = mixpool.tile([P, 4, T], bf16, tag="mx", name="u")
                    nc.vector.tensor_mul(u[:], t2[:], dpm_sb[:])
                    e2 = mixpool.tile([P, 4, T], bf16, tag="mx", name="e2")
                    nc.vector.tensor_add(e2[:], u[:], pdm_sb[:])
                    mixd = mixpool.tile([P, 4, T], bf16, tag="mx",
                                        name="mixd")
                    nc.vector.tensor_mul(mixd[:], A[:], e2[:])
                    nc.scalar.activation(E[:], mixd[:], AF.Exp)
                    E_pair.append(E)
                    del u, e2, mixd
                return E_pair

            def ctx_phase(pc, E_pair):
                for hh in range(2):
                    h = 2 * pc + hh
                    E = E_pair[hh]
                    cp = psA.tile([P, T], f32, tag="ps", name="ctx")
                    for j in range(ST):
                        nc.tensor.matmul(cp[0:DH + 1, :],
                                         lhsT=vaug[j][:, h, :],
                                         rhs=E[:, j, :],
                                         start=(j == 0), stop=(j == ST - 1))
                    co = opool.tile([DH + 1, T], bf16, tag="o", name="o")
                    nc.vector.tensor_copy(co[:], cp[0:DH + 1, :])
                    nc.sync.dma_start(out=out_ct[h], in_=co[:])

            # depth-2 software pipeline over head pairs.  Order within an
            # iteration keeps the TensorE FIFO free of head-of-line waits:
            # ctx(pc-2) right after scores (E ready), stats MMs between
            # MLP1 and MLP2 to cover the ti->tanh latency.
            NP = H // 2
            V_SCHED = {0: [(0, 0), (1, 0)], 1: [(2, 0), (3, 0)],
                       2: [(0, 1), (1, 1)], 3: [(2, 1), (3, 1)]}
            state = {}
            qk_proj(0)
            for pc in range(NP + 1):
                A_pair = None
                if pc < NP:
                    A_pair = scores_mm_phase(pc)
                if pc + 1 < NP:
                    qk_proj(pc + 1)
                for tt, ch in V_SCHED.get(pc, []):
                    v_proj(tt, ch)
                if pc >= 2:
                    ctx_phase(pc - 2, state[pc - 2][0]["E"])
                    del state[pc - 2]
                if 1 <= pc <= NP:
                    st, rs2f = state[pc - 1]
                    th_pair = gate_mlp1(st, rs2f)
                if pc < NP:
                    state[pc] = prep_phase(A_pair)
                if 1 <= pc <= NP:
                    t2_pair = gate_mlp2(th_pair)
                    E_pair = gate_b(st, t2_pair)
                    state[pc - 1] = ({"E": E_pair}, None)
                    if pc == NP:
                        ctx_phase(pc - 1, E_pair)

    nc.compile()
    return nc


def _prep(inputs):
    bfloat16 = ml_dtypes.bfloat16
    f8np = ml_dtypes.float8_e4m3
    hidden = np.asarray(inputs["hidden_states"], dtype=np.float32)
    mask = np.asarray(inputs["attention_mask"], dtype=np.float32)
    dep = np.asarray(inputs["dependency_matrix"], dtype=np.float32)
    ws = {k: np.asarray(inputs[k], dtype=np.float32)
          for k in ("Wq", "Wk", "Wv", "W1", "W2")}
    vs = {k: np.asarray(inputs[k], dtype=np.float32)
          for k in ("bq", "bk", "bv", "b1", "b2", "ln_g", "ln_b")}

    flags = {
        "bq": bool(np.any(vs["bq"])), "bk": bool(np.any(vs["bk"])),
        "bv": bool(np.any(vs["bv"])),
        "lng": bool(np.any(vs["ln_g"] != 1.0)),
        "c": bool(np.any(vs["ln_b"]) or np.any(vs["b1"])),
        "b2": bool(np.any(vs["b2"])),
        "mask": bool(np.any(mask != 1.0)),
    }
    if any(flags.values()):
        raise NotImplementedError(f"nontrivial flags unsupported: {flags}")

    wq_b = np.ascontiguousarray((ws["Wq"] * np.float32(0.125)).astype(bfloat16))
    wk_b = np.ascontiguousarray(ws["Wk"].astype(bfloat16))
    wv_b = np.ascontiguousarray(ws["Wv"].astype(bfloat16))

    # W1 * 64 -> f8, DoubleRow packed: w18[ki, p, ko, m] = W1[(2p+ko)*128+ki, m]
    w1s = (ws["W1"] * np.float32(64.0)).astype(f8np)
    w18 = np.ascontiguousarray(
        w1s.reshape(4, 2, P, TM).transpose(2, 0, 1, 3))
    # rs is applied as rs/64 on host side scale? No: fold 1/64 into c_rep
    # (rs = (ms*c)^-1/2 scales ti; W1 is 64x, so ti needs rs/64 ->
    #  equivalently c_rep *= 64^2).
    w2s = (ws["W2"] * np.float32(16.0)).astype(f8np)
    w2_b = np.ascontiguousarray(
        w2s.reshape(2, 2, P, TM).transpose(2, 0, 1, 3))

    in_maps = []
    for b in range(N_CORES):
        dt = dep[b].T  # dep^T[s, t] = dep[t, s]
        c = (1.0 + (dt * dt).mean(axis=0)) / (2 * TM)   # [T]
        c = c * np.float32(64.0 * 64.0)                  # fold W1 scale^2
        c_rep = np.ascontiguousarray(
            np.broadcast_to(c[None, :], (P, T)).astype(bfloat16))
        m = {
            "hid_t": np.ascontiguousarray(hidden[b].T.astype(bfloat16)),
            "dep_t": np.ascontiguousarray(dt.astype(bfloat16)),
            "crep_t": c_rep,
            "wq": wq_b, "wk": wk_b, "wv": wv_b,
            "w18": w18, "w2p": w2_b,
        }
        in_maps.append(m)
    return flags, in_maps


def kernel(**inputs):
    from concourse.bass_utils import run_bass_kernel_spmd

    flags, in_maps = _prep(inputs)
    nc = _build(flags)
    res = run_bass_kernel_spmd(nc, in_maps, core_ids=list(range(N_CORES)))
    out = np.empty((B, T, C), dtype=np.float32)
    for b, r in enumerate(res.results):
        oc = np.asarray(r["out_ct"], dtype=np.float32)  # [H, DH+1, T]
        ctx = oc[:, :DH, :] / oc[:, DH:DH + 1, :]
        out[b] = ctx.transpose(2, 0, 1).reshape(T, C)
    return out


# revision 60
# speedup vs baseline: 1.0051x; 1.0004x over previous
"""Trainium2 Bass kernel for nn_DependencyBertMix (v5).

Contract: kernel(**inputs) takes the FULL unsharded inputs (as produced by
setup_inputs()) and returns the FULL [8, 512, 768] float32 output.

Strategy: data-parallel over batch - B=8 batch elements, one per NeuronCore.
Weights replicated; no collectives.

Per-core pipeline in transposed [feature, t] layout (t = query, s = key):

  Q'_T = (Wq/8)^T @ hid_T    K_T likewise (V kept natural [t, c] with an
  appended ones column -> ctx matmul also accumulates the softmax denom).
  QK projections are emitted per-128-column group, interleaved into the
  head loop so the DMA/projection prolog overlaps the first head pairs.

  per head pair (h0, h1) - scores packed 2-per-PE-pass via tile_position
  rows 0-63 / 64-127 (contraction dim is DH=64):
    A_T[s,t]  = K_h as lhsT @ Q'_h        (bf16 [P,4,T], scalar copies)
    A8 = fp8(A) (DVE cast)   D8 = fp8(A (.) dep^T) (DVE mul, fp8 out)
    sq8 = fp8(A^2)           (scalar Square - same act table as tanh/exp)
    ms  = DoubleRow-ones @ sq8-pairs      (column sums, replicated rows)
    rs  = rsqrt(ms (.) c_rep) via the fp32 bit-trick seed only (max err
          3.4%, validated end-to-end); c_rep[t] = (1+mean_s dep^2)/(2T)
          * 64^2 folds the W1 fp8 scale; LN mean term dropped (|mu| ~
          0.03 sigma).  No scalar Sqrt -> zero act-table swaps.
    Y   = (64 W1)fp8^T @ [A8;D8]   DoubleRow, nt-outer with both heads
          sharing each weight load;  ti = Y (.) rs pair-multiplied.
    th  = tanh(ti) -> fp8;  G = (16 W2)fp8^T @ th-pairs (DoubleRow)
    t2  = tanh(G/32)                (sigmoid(x) = .5 + .5 tanh(x/2))
    mixd = A (.) ((1+dep)/2 + t2 (.) (1-dep)/2)  == g*A + (1-g)*D
    E   = exp(mixd)
    ctx^T[d,t] = sum_j vaug_j^T @ E_j  (4 MMs N=512; row 64 = denom).
    psum -> SBUF copy -> DMA; normalization + transpose happen on host.

Emission is a depth-2 software pipeline over head pairs ordered to keep
every engine FIFO free of head-of-line waits: scores(pc) -> ctx(pc-2) ->
MLP1(pc-1) -> fp8 prep/stats(pc) -> MLP2(pc-1) -> mix/exp(pc-1).
dpm/pdm are computed on-device from dep (affine).  ~185 us/core at the
fast clock state vs 269 us for the bf16 baseline.
"""
import sys

for _p in ("/opt/trn_rl_repo", "/opt/pypackages"):
    if _p not in sys.path:
        sys.path.append(_p)

import ml_dtypes
import numpy as np

B, T, C = 8, 512, 768
H, DH = 12, 64
TM = 512
N_CORES = 8
P = 128
ST = T // P          # 4 s-tiles
CO = C // P          # 6
MAGIC = 0x5F3759DF
USE_NR = False       # extra Newton step for rsqrt


def _build(flags):
    import concourse.tile as tile
    from concourse import bacc, mybir

    f32 = mybir.dt.float32
    bf16 = mybir.dt.bfloat16
    f8 = mybir.dt.float8e4
    i32 = mybir.dt.int32
    AF = mybir.ActivationFunctionType
    OP = mybir.AluOpType
    DR = mybir.MatmulPerfMode.DoubleRow

    nc = bacc.Bacc("TRN2", target_bir_lowering=False, debug=False,
                   enable_asserts=False, num_devices=N_CORES)

    # ---- DRAM I/O ----
    hid_t = nc.dram_tensor("hid_t", [C, T], bf16, kind="ExternalInput")
    dep_t = nc.dram_tensor("dep_t", [T, T], bf16, kind="ExternalInput")   # dep^T
    crep_t = nc.dram_tensor("crep_t", [P, T], bf16, kind="ExternalInput")
    wq = nc.dram_tensor("wq", [C, C], bf16, kind="ExternalInput")  # pre /8
    wk = nc.dram_tensor("wk", [C, C], bf16, kind="ExternalInput")
    wv = nc.dram_tensor("wv", [C, C], bf16, kind="ExternalInput")
    w18_d = nc.dram_tensor("w18", [P, 4, 2, TM], f8, kind="ExternalInput")
    w2p = nc.dram_tensor("w2p", [P, 2, 2, TM], f8, kind="ExternalInput")
    out_ct = nc.dram_tensor("out_ct", [H, DH + 1, T], bf16,
                            kind="ExternalOutput")

    with tile.TileContext(nc, pool_alloc_mode='queue') as tc:
        from contextlib import ExitStack
        with ExitStack() as _es:
            singles = _es.enter_context(tc.tile_pool(name="singles", bufs=1))
            wpool = _es.enter_context(tc.tile_pool(name="wpool", bufs=18))
            hidpool = _es.enter_context(tc.tile_pool(name="hidpool", bufs=6))
            apool = _es.enter_context(tc.tile_pool(name="apool", bufs=5))
            q8pool = _es.enter_context(tc.tile_pool(name="q8pool", bufs=10))
            rspool = _es.enter_context(tc.tile_pool(name="rspool", bufs=4))
            tipool = _es.enter_context(tc.tile_pool(name="tipool", bufs=4))
            thpool = _es.enter_context(tc.tile_pool(name="thpool", bufs=3))
            t2pool = _es.enter_context(tc.tile_pool(name="t2pool", bufs=3))
            mixpool = _es.enter_context(tc.tile_pool(name="mixpool", bufs=4))
            epool = _es.enter_context(tc.tile_pool(name="epool", bufs=5))
            opool = _es.enter_context(tc.tile_pool(name="opool", bufs=4))
            psA = _es.enter_context(tc.tile_pool(name="psA", bufs=4,
                                                 space="PSUM"))
            psB = _es.enter_context(tc.tile_pool(name="psB", bufs=2,
                                                 space="PSUM"))

            # ---------- early DMAs ----------
            hid_l = [hidpool.tile([P, T], bf16, tag="hid", name=f"hid{ci}")
                     for ci in range(CO)]

            ones8 = singles.tile([P, 2, P], f8, tag="ones8")
            nc.vector.memset(ones8[:], 1.0)

            # ---------- Q/K projections (transposed layout) ----------
            QT = singles.tile([P, CO, T], bf16, tag="QT")
            KT = singles.tile([P, CO, T], bf16, tag="KT")
            wq_l = [wpool.tile([P, C], bf16, tag="w", name="w")
                    for _ in range(CO)]
            wk_l = [wpool.tile([P, C], bf16, tag="w", name="w")
                    for _ in range(CO)]
            # interleave hid/wq/wk DMAs per-ci so the first projection
            # matmul (needs only hid[0]+wq[0]) starts ~4us earlier
            for ci in range(CO):
                nc.sync.dma_start(out=hid_l[ci][:],
                                  in_=hid_t[ci * P:(ci + 1) * P, :])
                nc.sync.dma_start(out=wq_l[ci][:],
                                  in_=wq[ci * P:(ci + 1) * P, :])
                nc.sync.dma_start(out=wk_l[ci][:],
                                  in_=wk[ci * P:(ci + 1) * P, :])

            def qk_proj(cot):
                for w_l, dest in ((wq_l, QT), (wk_l, KT)):
                    pp = psA.tile([P, T], f32, tag="ps", name="pp")
                    for ci in range(CO):
                        nc.tensor.matmul(
                            pp[:],
                            lhsT=w_l[ci][:, cot * P:(cot + 1) * P],
                            rhs=hid_l[ci][:],
                            start=(ci == 0), stop=(ci == CO - 1))
                    nc.scalar.copy(dest[:, cot, :], pp[:])

            # ---------- V projection (natural layout + ones column) ----------
            vaug = [singles.tile([P, H, DH + 1], bf16, tag=f"v{i}",
                                 name=f"v{i}") for i in range(ST)]
            for tt in range(ST):
                nc.vector.memset(vaug[tt][:, :, DH:DH + 1], 1.0)
            w_l = [wpool.tile([P, C], bf16, tag="w", name="w")
                   for _ in range(CO)]
            for ci in range(CO):
                nc.sync.dma_start(out=w_l[ci][:], in_=wv[ci * P:(ci + 1) * P, :])
            CHW = C // 2  # 384

            def v_proj(tt, ch):
                vp = psA.tile([P, T], f32, tag="ps", name="vp")
                for ci in range(CO):
                    nc.tensor.matmul(
                        vp[:, 0:CHW],
                        lhsT=hid_l[ci][:, tt * P:(tt + 1) * P],
                        rhs=w_l[ci][:, ch * CHW:(ch + 1) * CHW],
                        start=(ci == 0), stop=(ci == CO - 1))
                nc.scalar.copy(
                    vaug[tt][:, ch * 6:(ch + 1) * 6, 0:DH],
                    vp[:, 0:CHW].rearrange("p (h d) -> p h d", d=DH))

            # ---------- late DMAs ----------
            dep_sb = singles.tile([P, ST, T], bf16, tag="dep")
            for j in range(ST):
                nc.sync.dma_start(out=dep_sb[:, j, :],
                                  in_=dep_t[j * P:(j + 1) * P, :])
            dpm_sb = singles.tile([P, ST, T], bf16, tag="dpm")
            nc.vector.tensor_scalar(dpm_sb[:], dep_sb[:], -0.5, 0.5,
                                    op0=OP.mult, op1=OP.add)
            pdm_sb = singles.tile([P, ST, T], bf16, tag="pdm")
            nc.vector.tensor_scalar(pdm_sb[:], dep_sb[:], 0.5, 0.5,
                                    op0=OP.mult, op1=OP.add)
            crep = singles.tile([P, T], bf16, tag="crep")
            nc.sync.dma_start(out=crep[:], in_=crep_t[:])
            w18_sb = singles.tile([P, 4, 2, TM], f8, tag="w18")
            nc.sync.dma_start(out=w18_sb[:], in_=w18_d[:])
            w2_sb = singles.tile([P, 2, 2, TM], f8, tag="w2")
            nc.sync.dma_start(out=w2_sb[:], in_=w2p[:])

            # ---------- per-head-pair phases ----------
            def scores_mm_phase(pc):
                """Paired scores MMs + A copies."""
                A_pair = []
                for hh in range(2):
                    A_pair.append(apool.tile([P, ST, T], bf16, tag="A",
                                             name="A"))
                for j in range(ST):
                    sp0 = psA.tile([P, T], f32, tag="ps", name="sp0")
                    sp1 = psA.tile([P, T], f32, tag="ps", name="sp1")
                    jsl = slice(j * P, (j + 1) * P)
                    nc.tensor.matmul(sp0[:], lhsT=KT[0:DH, pc, jsl],
                                     rhs=QT[0:DH, pc, :],
                                     start=True, stop=True,
                                     tile_position=(0, 0))
                    nc.tensor.matmul(sp1[:], lhsT=KT[DH:P, pc, jsl],
                                     rhs=QT[DH:P, pc, :],
                                     start=True, stop=True,
                                     tile_position=(64, 0))
                    nc.scalar.copy(A_pair[0][:, j, :], sp0[:])
                    nc.scalar.copy(A_pair[1][:, j, :], sp1[:])
                return A_pair

            def prep_phase(A_pair):
                """fp8 prep + stats -> rs (consumed next stage)."""
                out = []
                q2 = rspool.tile([P, 2, T], f32, tag="rs", name="q2")
                prep = []
                for hh in range(2):
                    A = A_pair[hh]
                    A8 = q8pool.tile([P, ST, T], f8, tag="q8", name="A8")
                    nc.vector.tensor_copy(A8[:], A[:])
                    D8 = q8pool.tile([P, ST, T], f8, tag="q8", name="D8")
                    nc.vector.tensor_mul(D8[:], A[:], dep_sb[:])
                    sq8 = q8pool.tile([P, ST, T], f8, tag="q8", name="sq8")
                    nc.scalar.activation(sq8[:], A[:], AF.Square)
                    ms_ps = psA.tile([P, T], f32, tag="ps", name="ms")
                    for p in range(2):
                        nc.tensor.matmul(ms_ps[:], lhsT=ones8[:],
                                         rhs=sq8[:, 2 * p:2 * p + 2, :],
                                         start=(p == 0), stop=(p == 1),
                                         perf_mode=DR)
                    nc.vector.tensor_mul(q2[:, hh, :], ms_ps[:], crep[:])
                    prep.append((A, A8, D8))
                r_i = rspool.tile([P, 2, T], i32, tag="rs", name="ri")
                nc.vector.tensor_scalar(r_i[:], q2[:].bitcast(i32), 1,
                                        None, op0=OP.arith_shift_right)
                nc.vector.tensor_scalar(r_i[:], r_i[:], MAGIC, -1,
                                        op0=OP.subtract, op1=OP.mult)
                for hh in range(2):
                    A, A8, D8 = prep[hh]
                    out.append((A, A8, D8, r_i[:, hh, :].bitcast(f32)))
                return out, r_i[:].bitcast(f32)

            def gate_mlp1(state, rs2f):
                """MLP1 nt-outer with both heads sharing weight loads;
                ti pair-multiplied by rs2; tanh per (nt, head)."""
                th_pair = [thpool.tile([P, 4, T], f8, tag="th", name="th")
                           for _ in range(2)]
                for nt in range(4):
                    nsl = slice(nt * P, (nt + 1) * P)
                    y_ps = psB.tile([P, 2, T], f32, tag="pb", name="y")
                    for p in range(4):
                        for hh in range(2):
                            (A, A8, D8, rs) = state[hh]
                            src = A8 if p < 2 else D8
                            q = p % 2
                            nc.tensor.matmul(
                                y_ps[:, hh, :],
                                lhsT=w18_sb[:, p, :, nsl],
                                rhs=src[:, 2 * q:2 * q + 2, :],
                                start=(p == 0), stop=(p == 3),
                                perf_mode=DR)
                    ti = tipool.tile([P, 2, T], bf16, tag="ti", name="ti")
                    nc.vector.tensor_mul(ti[:], y_ps[:], rs2f)
                    for hh in range(2):
                        nc.scalar.activation(th_pair[hh][:, nt, :],
                                             ti[:, hh, :], AF.Tanh)
                return th_pair

            def gate_mlp2(th_pair):
                """MLP2 -> t2, per head."""
                t2_pair = []
                for th in th_pair:
                    t2 = t2pool.tile([P, 4, T], bf16, tag="t2", name="t2")
                    for ni in range(2):
                        g_ps = psB.tile([P, 2, T], f32, tag="pb", name="g")
                        for sub in range(2):
                            nt = 2 * ni + sub
                            nsl = slice(nt * P, (nt + 1) * P)
                            for m in range(2):
                                nc.tensor.matmul(
                                    g_ps[:, sub, :],
                                    lhsT=w2_sb[:, m, :, nsl],
                                    rhs=th[:, 2 * m:2 * m + 2, :],
                                    start=(m == 0), stop=(m == 1),
                                    perf_mode=DR)
                        nc.scalar.activation(t2[:, 2 * ni:2 * ni + 2, :],
                                             g_ps[:], AF.Tanh,
                                             scale=0.03125)
                    t2_pair.append(t2)
                return t2_pair

            def gate_b(state, t2_pair):
                """mix + exp in halves, per head."""
                E_pair = []
                for hh in range(2):
                    (A, A8, D8, rs) = state[hh]
                    t2 = t2_pair[hh]
                    E = epool.tile([P, 4, T], bf16, tag="E", name="E")
                    u = mixpool.tile([P, 4, T], bf16, tag="mx", name="u")
                    nc.vector.tensor_mul(u[:], t2[:], dpm_sb[:])
                    e2 = mixpool.tile([P, 4, T], bf16, tag="mx", name="e2")
                    nc.vector.tensor_add(e2[:], u[:], pdm_sb[:])
                    mixd = mixpool.tile([P, 4, T], bf16, tag="mx",
                                        name="mixd")
                    nc.vector.tensor_mul(mixd[:], A[:], e2[:])
                    nc.scalar.activation(E[:], mixd[:], AF.Exp)
                    E_pair.append(E)
                    del u, e2, mixd
                return E_pair

            def ctx_phase(pc, E_pair):
                for hh in range(2):
                    h = 2 * pc + hh
                    E = E_pair[hh]
                    cp = psA.tile([P, T], f32, tag="ps", name="ctx")
                    for j in range(ST):
                        nc.tensor.matmul(cp[0:DH + 1, :],
                                         lhsT=vaug[j][:, h, :],
                                         rhs=E[:, j, :],
                                         start=(j == 0), stop=(j == ST - 1))
                    co = opool.tile([DH + 1, T], bf16, tag="o", name="o")
                    nc.vector.tensor_copy(co[:], cp[0:DH + 1, :])
                    nc.sync.dma_start(out=out_ct[h], in_=co[:])

            # depth-2 software pipeline over head pairs.  Order within an
            # iteration keeps the TensorE FIFO free of head-of-line waits:
            # ctx(pc-2) right after scores (E ready), stats MMs between
            # MLP1 and MLP2 to cover the ti->tanh latency.
            NP = H // 2
            V_SCHED = {0: [(0, 0), (1, 0)], 1: [(2, 0), (3, 0)],
                       2: [(0, 1), (1, 1)], 3: [(2, 1), (3, 1)]}
            state = {}
            qk_proj(0)
            for pc in range(NP + 1):
                A_pair = None
                if pc < NP:
                    A_pair = scores_mm_phase(pc)
                if pc + 1 < NP:
                    qk_proj(pc + 1)
                for tt, ch in V_SCHED.get(pc, []):
                    v_proj(tt, ch)
                if pc >= 2:
                    ctx_phase(pc - 2, state[pc - 2][0]["E"])
                    del state[pc - 2]
                if 1 <= pc <= NP:
                    st, rs2f = state[pc - 1]
                    th_pair = gate_mlp1(st, rs2f)
                if pc < NP:
                    state[pc] = prep_phase(A_pair)
                if 1 <= pc <= NP:
                    t2_pair = gate_mlp2(th_pair)
                    E_pair = gate_b(st, t2_pair)
                    state[pc - 1] = ({"E": E_pair}, None)
                    if pc == NP:
                        ctx_phase(pc - 1, E_pair)

    nc.compile()
    return nc


def _prep(inputs):
    bfloat16 = ml_dtypes.bfloat16
    f8np = ml_dtypes.float8_e4m3
    hidden = np.asarray(inputs["hidden_states"], dtype=np.float32)
    mask = np.asarray(inputs["attention_mask"], dtype=np.float32)
    dep = np.asarray(inputs["dependency_matrix"], dtype=np.float32)
    ws = {k: np.asarray(inputs[k], dtype=np.float32)
          for k in ("Wq", "Wk", "Wv", "W1", "W2")}
    vs = {k: np.asarray(inputs[k], dtype=np.float32)
          for k in ("bq", "bk", "bv", "b1", "b2", "ln_g", "ln_b")}

    flags = {
        "bq": bool(np.any(vs["bq"])), "bk": bool(np.any(vs["bk"])),
        "bv": bool(np.any(vs["bv"])),
        "lng": bool(np.any(vs["ln_g"] != 1.0)),
        "c": bool(np.any(vs["ln_b"]) or np.any(vs["b1"])),
        "b2": bool(np.any(vs["b2"])),
        "mask": bool(np.any(mask != 1.0)),
    }
    if any(flags.values()):
        raise NotImplementedError(f"nontrivial flags unsupported: {flags}")

    wq_b = np.ascontiguousarray((ws["Wq"] * np.float32(0.125)).astype(bfloat16))
    wk_b = np.ascontiguousarray(ws["Wk"].astype(bfloat16))
    wv_b = np.ascontiguousarray(ws["Wv"].astype(bfloat16))

    # W1 * 64 -> f8, DoubleRow packed: w18[ki, p, ko, m] = W1[(2p+ko)*128+ki, m]
    w1s = (ws["W1"] * np.float32(64.0)).astype(f8np)
    w18 = np.ascontiguousarray(
        w1s.reshape(4, 2, P, TM).transpose(2, 0, 1, 3))
    # rs is applied as rs/64 on host side scale? No: fold 1/64 into c_rep
    # (rs = (ms*c)^-1/2 scales ti; W1 is 64x, so ti needs rs/64 ->
    #  equivalently c_rep *= 64^2).
    w2s = (ws["W2"] * np.float32(16.0)).astype(f8np)
    w2_b = np.ascontiguousarray(
        w2s.reshape(2, 2, P, TM).transpose(2, 0, 1, 3))

    in_maps = []
    for b in range(N_CORES):
        dt = dep[b].T  # dep^T[s, t] = dep[t, s]
        c = (1.0 + (dt * dt).mean(axis=0)) / (2 * TM)   # [T]
        c = c * np.float32(64.0 * 64.0)                  # fold W1 scale^2
        c_rep = np.ascontiguousarray(
            np.broadcast_to(c[None, :], (P, T)).astype(bfloat16))
        m = {
            "hid_t": np.ascontiguousarray(hidden[b].T.astype(bfloat16)),
            "dep_t": np.ascontiguousarray(dt.astype(bfloat16)),
            "crep_t": c_rep,
            "wq": wq_b, "wk": wk_b, "wv": wv_b,
            "w18": w18, "w2p": w2_b,
        }
        in_maps.append(m)
    return flags, in_maps


def kernel(**inputs):
    from concourse.bass_utils import run_bass_kernel_spmd

    flags, in_maps = _prep(inputs)
    nc = _build(flags)
    res = run_bass_kernel_spmd(nc, in_maps, core_ids=list(range(N_CORES)))
    out = np.empty((B, T, C), dtype=np.float32)
    for b, r in enumerate(res.results):
        oc = np.asarray(r["out_ct"], dtype=np.float32)  # [H, DH+1, T]
        ctx = oc[:, :DH, :] / oc[:, DH:DH + 1, :]
        out[b] = ctx.transpose(2, 0, 1).reshape(T, C)
    return out


# revision 61
# speedup vs baseline: 1.0221x; 1.0170x over previous
"""Trainium2 Bass kernel for nn_DependencyBertMix (v5).

Contract: kernel(**inputs) takes the FULL unsharded inputs (as produced by
setup_inputs()) and returns the FULL [8, 512, 768] float32 output.

Strategy: data-parallel over batch - B=8 batch elements, one per NeuronCore.
Weights replicated; no collectives.

Per-core pipeline in transposed [feature, t] layout (t = query, s = key):

  Q'_T = (Wq/8)^T @ hid_T    K_T likewise (V kept natural [t, c] with an
  appended ones column -> ctx matmul also accumulates the softmax denom).
  QK projections are emitted per-128-column group, interleaved into the
  head loop so the DMA/projection prolog overlaps the first head pairs.

  per head pair (h0, h1) - scores packed 2-per-PE-pass via tile_position
  rows 0-63 / 64-127 (contraction dim is DH=64):
    A_T[s,t]  = K_h as lhsT @ Q'_h        (bf16 [P,4,T], scalar copies)
    A8 = fp8(A) (DVE cast)   D8 = fp8(A (.) dep^T) (DVE mul, fp8 out)
    sq8 = fp8(A^2)           (scalar Square - same act table as tanh/exp)
    ms  = DoubleRow-ones @ sq8-pairs      (column sums, replicated rows)
    rs  = rsqrt(ms (.) c_rep) via the fp32 bit-trick seed only (max err
          3.4%, validated end-to-end); c_rep[t] = (1+mean_s dep^2)/(2T)
          * 64^2 folds the W1 fp8 scale; LN mean term dropped (|mu| ~
          0.03 sigma).  No scalar Sqrt -> zero act-table swaps.
    Y   = (64 W1)fp8^T @ [A8;D8]   DoubleRow, nt-outer with both heads
          sharing each weight load;  ti = Y (.) rs pair-multiplied.
    th  = tanh(ti) -> fp8;  G = (16 W2)fp8^T @ th-pairs (DoubleRow)
    t2  = tanh(G/32)                (sigmoid(x) = .5 + .5 tanh(x/2))
    mixd = A (.) ((1+dep)/2 + t2 (.) (1-dep)/2)  == g*A + (1-g)*D
    E   = exp(mixd)
    ctx^T[d,t] = sum_j vaug_j^T @ E_j  (4 MMs N=512; row 64 = denom).
    psum -> SBUF copy -> DMA; normalization + transpose happen on host.

Emission is a depth-2 software pipeline over head pairs ordered to keep
every engine FIFO free of head-of-line waits: scores(pc) -> ctx(pc-2) ->
MLP1(pc-1) -> fp8 prep/stats(pc) -> MLP2(pc-1) -> mix/exp(pc-1).
dpm/pdm are computed on-device from dep (affine).  ~185 us/core at the
fast clock state vs 269 us for the bf16 baseline.
"""
import sys

for _p in ("/opt/trn_rl_repo", "/opt/pypackages"):
    if _p not in sys.path:
        sys.path.append(_p)

import ml_dtypes
import numpy as np

B, T, C = 8, 512, 768
H, DH = 12, 64
TM = 512
N_CORES = 8
P = 128
ST = T // P          # 4 s-tiles
CO = C // P          # 6
MAGIC = 0x5F3759DF
USE_NR = False       # extra Newton step for rsqrt


def _build(flags):
    import concourse.tile as tile
    from concourse import bacc, mybir

    f32 = mybir.dt.float32
    bf16 = mybir.dt.bfloat16
    f8 = mybir.dt.float8e4
    i32 = mybir.dt.int32
    AF = mybir.ActivationFunctionType
    OP = mybir.AluOpType
    DR = mybir.MatmulPerfMode.DoubleRow

    nc = bacc.Bacc("TRN2", target_bir_lowering=False, debug=False,
                   enable_asserts=False, num_devices=N_CORES)

    # ---- DRAM I/O ----
    hid_t = nc.dram_tensor("hid_t", [C, T], bf16, kind="ExternalInput")
    dep_t = nc.dram_tensor("dep_t", [T, T], bf16, kind="ExternalInput")   # dep^T
    crep_t = nc.dram_tensor("crep_t", [P, T], bf16, kind="ExternalInput")
    wq = nc.dram_tensor("wq", [C, C], bf16, kind="ExternalInput")  # pre /8
    wk = nc.dram_tensor("wk", [C, C], bf16, kind="ExternalInput")
    wv = nc.dram_tensor("wv", [C, C], bf16, kind="ExternalInput")
    w18_d = nc.dram_tensor("w18", [P, 4, 2, TM], f8, kind="ExternalInput")
    w2p = nc.dram_tensor("w2p", [P, 2, 2, TM], f8, kind="ExternalInput")
    out_ct = nc.dram_tensor("out_ct", [H, DH + 1, T], bf16,
                            kind="ExternalOutput")

    with tile.TileContext(nc) as tc:
        from contextlib import ExitStack
        with ExitStack() as _es:
            singles = _es.enter_context(tc.tile_pool(name="singles", bufs=1))
            wpool = _es.enter_context(tc.tile_pool(name="wpool", bufs=18))
            hidpool = _es.enter_context(tc.tile_pool(name="hidpool", bufs=6))
            apool = _es.enter_context(tc.tile_pool(name="apool", bufs=5))
            q8pool = _es.enter_context(tc.tile_pool(name="q8pool", bufs=10))
            rspool = _es.enter_context(tc.tile_pool(name="rspool", bufs=4))
            tipool = _es.enter_context(tc.tile_pool(name="tipool", bufs=4))
            thpool = _es.enter_context(tc.tile_pool(name="thpool", bufs=3))
            t2pool = _es.enter_context(tc.tile_pool(name="t2pool", bufs=3))
            mixpool = _es.enter_context(tc.tile_pool(name="mixpool", bufs=4))
            epool = _es.enter_context(tc.tile_pool(name="epool", bufs=5))
            opool = _es.enter_context(tc.tile_pool(name="opool", bufs=4))
            psA = _es.enter_context(tc.tile_pool(name="psA", bufs=4,
                                                 space="PSUM"))
            psB = _es.enter_context(tc.tile_pool(name="psB", bufs=2,
                                                 space="PSUM"))

            # ---------- early DMAs ----------
            hid_l = [hidpool.tile([P, T], bf16, tag="hid", name=f"hid{ci}")
                     for ci in range(CO)]

            ones8 = singles.tile([P, 2, P], f8, tag="ones8")
            nc.vector.memset(ones8[:], 1.0)

            # ---------- Q/K projections (transposed layout) ----------
            QT = singles.tile([P, CO, T], bf16, tag="QT")
            KT = singles.tile([P, CO, T], bf16, tag="KT")
            wq_l = [wpool.tile([P, C], bf16, tag="w", name="w")
                    for _ in range(CO)]
            wk_l = [wpool.tile([P, C], bf16, tag="w", name="w")
                    for _ in range(CO)]
            # interleave hid/wq/wk DMAs per-ci so the first projection
            # matmul (needs only hid[0]+wq[0]) starts ~4us earlier
            for ci in range(CO):
                nc.sync.dma_start(out=hid_l[ci][:],
                                  in_=hid_t[ci * P:(ci + 1) * P, :])
                nc.sync.dma_start(out=wq_l[ci][:],
                                  in_=wq[ci * P:(ci + 1) * P, :])
                nc.sync.dma_start(out=wk_l[ci][:],
                                  in_=wk[ci * P:(ci + 1) * P, :])

            def qk_proj(cot):
                for w_l, dest in ((wq_l, QT), (wk_l, KT)):
                    pp = psA.tile([P, T], f32, tag="ps", name="pp")
                    for ci in range(CO):
                        nc.tensor.matmul(
                            pp[:],
                            lhsT=w_l[ci][:, cot * P:(cot + 1) * P],
                            rhs=hid_l[ci][:],
                            start=(ci == 0), stop=(ci == CO - 1))
                    nc.scalar.copy(dest[:, cot, :], pp[:])

            # ---------- V projection (natural layout + ones column) ----------
            vaug = [singles.tile([P, H, DH + 1], bf16, tag=f"v{i}",
                                 name=f"v{i}") for i in range(ST)]
            for tt in range(ST):
                nc.vector.memset(vaug[tt][:, :, DH:DH + 1], 1.0)
            w_l = [wpool.tile([P, C], bf16, tag="w", name="w")
                   for _ in range(CO)]
            for ci in range(CO):
                nc.sync.dma_start(out=w_l[ci][:], in_=wv[ci * P:(ci + 1) * P, :])
            CHW = C // 2  # 384

            def v_proj(tt, ch):
                vp = psA.tile([P, T], f32, tag="ps", name="vp")
                for ci in range(CO):
                    nc.tensor.matmul(
                        vp[:, 0:CHW],
                        lhsT=hid_l[ci][:, tt * P:(tt + 1) * P],
                        rhs=w_l[ci][:, ch * CHW:(ch + 1) * CHW],
                        start=(ci == 0), stop=(ci == CO - 1))
                nc.scalar.copy(
                    vaug[tt][:, ch * 6:(ch + 1) * 6, 0:DH],
                    vp[:, 0:CHW].rearrange("p (h d) -> p h d", d=DH))

            # ---------- late DMAs ----------
            dep_sb = singles.tile([P, ST, T], bf16, tag="dep")
            for j in range(ST):
                nc.sync.dma_start(out=dep_sb[:, j, :],
                                  in_=dep_t[j * P:(j + 1) * P, :])
            dpm_sb = singles.tile([P, ST, T], bf16, tag="dpm")
            nc.vector.tensor_scalar(dpm_sb[:], dep_sb[:], -0.5, 0.5,
                                    op0=OP.mult, op1=OP.add)
            pdm_sb = singles.tile([P, ST, T], bf16, tag="pdm")
            nc.vector.tensor_scalar(pdm_sb[:], dep_sb[:], 0.5, 0.5,
                                    op0=OP.mult, op1=OP.add)
            crep = singles.tile([P, T], bf16, tag="crep")
            nc.sync.dma_start(out=crep[:], in_=crep_t[:])
            w18_sb = singles.tile([P, 4, 2, TM], f8, tag="w18")
            nc.sync.dma_start(out=w18_sb[:], in_=w18_d[:])
            w2_sb = singles.tile([P, 2, 2, TM], f8, tag="w2")
            nc.sync.dma_start(out=w2_sb[:], in_=w2p[:])

            # ---------- per-head-pair phases ----------
            def scores_mm_phase(pc):
                """Paired scores MMs + A copies."""
                A_pair = []
                for hh in range(2):
                    A_pair.append(apool.tile([P, ST, T], bf16, tag="A",
                                             name="A"))
                for j in range(ST):
                    sp0 = psA.tile([P, T], f32, tag="ps", name="sp0")
                    sp1 = psA.tile([P, T], f32, tag="ps", name="sp1")
                    jsl = slice(j * P, (j + 1) * P)
                    nc.tensor.matmul(sp0[:], lhsT=KT[0:DH, pc, jsl],
                                     rhs=QT[0:DH, pc, :],
                                     start=True, stop=True,
                                     tile_position=(0, 0))
                    nc.tensor.matmul(sp1[:], lhsT=KT[DH:P, pc, jsl],
                                     rhs=QT[DH:P, pc, :],
                                     start=True, stop=True,
                                     tile_position=(64, 0))
                    nc.scalar.copy(A_pair[0][:, j, :], sp0[:])
                    nc.scalar.copy(A_pair[1][:, j, :], sp1[:])
                return A_pair

            def prep_phase(A_pair):
                """fp8 prep + stats -> rs (consumed next stage)."""
                out = []
                q2 = rspool.tile([P, 2, T], f32, tag="rs", name="q2")
                prep = []
                for hh in range(2):
                    A = A_pair[hh]
                    A8 = q8pool.tile([P, ST, T], f8, tag="q8", name="A8")
                    nc.vector.tensor_copy(A8[:], A[:])
                    D8 = q8pool.tile([P, ST, T], f8, tag="q8", name="D8")
                    nc.vector.tensor_mul(D8[:], A[:], dep_sb[:])
                    sq8 = q8pool.tile([P, ST, T], f8, tag="q8", name="sq8")
                    nc.scalar.activation(sq8[:], A[:], AF.Square)
                    ms_ps = psA.tile([P, T], f32, tag="ps", name="ms")
                    for p in range(2):
                        nc.tensor.matmul(ms_ps[:], lhsT=ones8[:],
                                         rhs=sq8[:, 2 * p:2 * p + 2, :],
                                         start=(p == 0), stop=(p == 1),
                                         perf_mode=DR)
                    nc.vector.tensor_mul(q2[:, hh, :], ms_ps[:], crep[:])
                    prep.append((A, A8, D8))
                r_i = rspool.tile([P, 2, T], i32, tag="rs", name="ri")
                nc.vector.tensor_scalar(r_i[:], q2[:].bitcast(i32), 1,
                                        None, op0=OP.arith_shift_right)
                nc.vector.tensor_scalar(r_i[:], r_i[:], MAGIC, -1,
                                        op0=OP.subtract, op1=OP.mult)
                for hh in range(2):
                    A, A8, D8 = prep[hh]
                    out.append((A, A8, D8, r_i[:, hh, :].bitcast(f32)))
                return out, r_i[:].bitcast(f32)

            def gate_mlp1(state, rs2f):
                """MLP1 nt-outer with both heads sharing weight loads;
                ti pair-multiplied by rs2; tanh per (nt, head)."""
                th_pair = [thpool.tile([P, 4, T], f8, tag="th", name="th")
                           for _ in range(2)]
                for nt in range(4):
                    nsl = slice(nt * P, (nt + 1) * P)
                    y_ps = psB.tile([P, 2, T], f32, tag="pb", name="y")
                    for p in range(4):
                        for hh in range(2):
                            (A, A8, D8, rs) = state[hh]
                            src = A8 if p < 2 else D8
                            q = p % 2
                            nc.tensor.matmul(
                                y_ps[:, hh, :],
                                lhsT=w18_sb[:, p, :, nsl],
                                rhs=src[:, 2 * q:2 * q + 2, :],
                                start=(p == 0), stop=(p == 3),
                                perf_mode=DR)
                    ti = tipool.tile([P, 2, T], bf16, tag="ti", name="ti")
                    nc.vector.tensor_mul(ti[:], y_ps[:], rs2f)
                    for hh in range(2):
                        nc.scalar.activation(th_pair[hh][:, nt, :],
                                             ti[:, hh, :], AF.Tanh)
                return th_pair

            def gate_mlp2(th_pair):
                """MLP2 -> t2, per head."""
                t2_pair = []
                for th in th_pair:
                    t2 = t2pool.tile([P, 4, T], bf16, tag="t2", name="t2")
                    for ni in range(2):
                        g_ps = psB.tile([P, 2, T], f32, tag="pb", name="g")
                        for sub in range(2):
                            nt = 2 * ni + sub
                            nsl = slice(nt * P, (nt + 1) * P)
                            for m in range(2):
                                nc.tensor.matmul(
                                    g_ps[:, sub, :],
                                    lhsT=w2_sb[:, m, :, nsl],
                                    rhs=th[:, 2 * m:2 * m + 2, :],
                                    start=(m == 0), stop=(m == 1),
                                    perf_mode=DR)
                        nc.scalar.activation(t2[:, 2 * ni:2 * ni + 2, :],
                                             g_ps[:], AF.Tanh,
                                             scale=0.03125)
                    t2_pair.append(t2)
                return t2_pair

            def gate_b(state, t2_pair):
                """mix + exp in halves, per head."""
                E_pair = []
                for hh in range(2):
                    (A, A8, D8, rs) = state[hh]
                    t2 = t2_pair[hh]
                    E = epool.tile([P, 4, T], bf16, tag="E", name="E")
                    u = mixpool.tile([P, 4, T], bf16, tag="mx", name="u")
                    nc.vector.tensor_mul(u[:], t2[:], dpm_sb[:])
                    e2 = mixpool.tile([P, 4, T], bf16, tag="mx", name="e2")
                    nc.vector.tensor_add(e2[:], u[:], pdm_sb[:])
                    mixd = mixpool.tile([P, 4, T], bf16, tag="mx",
                                        name="mixd")
                    nc.vector.tensor_mul(mixd[:], A[:], e2[:])
                    nc.scalar.activation(E[:], mixd[:], AF.Exp)
                    E_pair.append(E)
                    del u, e2, mixd
                return E_pair

            def ctx_phase(pc, E_pair):
                for hh in range(2):
                    h = 2 * pc + hh
                    E = E_pair[hh]
                    cp = psA.tile([P, T], f32, tag="ps", name="ctx")
                    for j in range(ST):
                        nc.tensor.matmul(cp[0:DH + 1, :],
                                         lhsT=vaug[j][:, h, :],
                                         rhs=E[:, j, :],
                                         start=(j == 0), stop=(j == ST - 1))
                    co = opool.tile([DH + 1, T], bf16, tag="o", name="o")
                    nc.vector.tensor_copy(co[:], cp[0:DH + 1, :])
                    nc.sync.dma_start(out=out_ct[h], in_=co[:])

            # depth-2 software pipeline over head pairs.  Order within an
            # iteration keeps the TensorE FIFO free of head-of-line waits:
            # ctx(pc-2) right after scores (E ready), stats MMs between
            # MLP1 and MLP2 to cover the ti->tanh latency.
            NP = H // 2
            V_SCHED = {0: [(0, 0), (1, 0)], 1: [(2, 0), (3, 0)],
                       2: [(0, 1), (1, 1)], 3: [(2, 1), (3, 1)]}
            state = {}
            qk_proj(0)
            for pc in range(NP + 1):
                A_pair = None
                if pc < NP:
                    A_pair = scores_mm_phase(pc)
                if pc + 1 < NP:
                    qk_proj(pc + 1)
                for tt, ch in V_SCHED.get(pc, []):
                    v_proj(tt, ch)
                if pc >= 2:
                    ctx_phase(pc - 2, state[pc - 2][0]["E"])
                    del state[pc - 2]
                if 1 <= pc <= NP:
                    st, rs2f = state[pc - 1]
                    th_pair = gate_mlp1(st, rs2f)
                if pc < NP:
                    state[pc] = prep_phase(A_pair)
                if 1 <= pc <= NP:
                    t2_pair = gate_mlp2(th_pair)
                    E_pair = gate_b(st, t2_pair)
                    state[pc - 1] = ({"E": E_pair}, None)
                    if pc == NP:
                        ctx_phase(pc - 1, E_pair)

    nc.compile()
    return nc


def _prep(inputs):
    bfloat16 = ml_dtypes.bfloat16
    f8np = ml_dtypes.float8_e4m3
    hidden = np.asarray(inputs["hidden_states"], dtype=np.float32)
    mask = np.asarray(inputs["attention_mask"], dtype=np.float32)
    dep = np.asarray(inputs["dependency_matrix"], dtype=np.float32)
    ws = {k: np.asarray(inputs[k], dtype=np.float32)
          for k in ("Wq", "Wk", "Wv", "W1", "W2")}
    vs = {k: np.asarray(inputs[k], dtype=np.float32)
          for k in ("bq", "bk", "bv", "b1", "b2", "ln_g", "ln_b")}

    flags = {
        "bq": bool(np.any(vs["bq"])), "bk": bool(np.any(vs["bk"])),
        "bv": bool(np.any(vs["bv"])),
        "lng": bool(np.any(vs["ln_g"] != 1.0)),
        "c": bool(np.any(vs["ln_b"]) or np.any(vs["b1"])),
        "b2": bool(np.any(vs["b2"])),
        "mask": bool(np.any(mask != 1.0)),
    }
    if any(flags.values()):
        raise NotImplementedError(f"nontrivial flags unsupported: {flags}")

    wq_b = np.ascontiguousarray((ws["Wq"] * np.float32(0.125)).astype(bfloat16))
    wk_b = np.ascontiguousarray(ws["Wk"].astype(bfloat16))
    wv_b = np.ascontiguousarray(ws["Wv"].astype(bfloat16))

    # W1 * 64 -> f8, DoubleRow packed: w18[ki, p, ko, m] = W1[(2p+ko)*128+ki, m]
    w1s = (ws["W1"] * np.float32(64.0)).astype(f8np)
    w18 = np.ascontiguousarray(
        w1s.reshape(4, 2, P, TM).transpose(2, 0, 1, 3))
    # rs is applied as rs/64 on host side scale? No: fold 1/64 into c_rep
    # (rs = (ms*c)^-1/2 scales ti; W1 is 64x, so ti needs rs/64 ->
    #  equivalently c_rep *= 64^2).
    w2s = (ws["W2"] * np.float32(16.0)).astype(f8np)
    w2_b = np.ascontiguousarray(
        w2s.reshape(2, 2, P, TM).transpose(2, 0, 1, 3))

    in_maps = []
    for b in range(N_CORES):
        dt = dep[b].T  # dep^T[s, t] = dep[t, s]
        c = (1.0 + (dt * dt).mean(axis=0)) / (2 * TM)   # [T]
        c = c * np.float32(64.0 * 64.0)                  # fold W1 scale^2
        c_rep = np.ascontiguousarray(
            np.broadcast_to(c[None, :], (P, T)).astype(bfloat16))
        m = {
            "hid_t": np.ascontiguousarray(hidden[b].T.astype(bfloat16)),
            "dep_t": np.ascontiguousarray(dt.astype(bfloat16)),
            "crep_t": c_rep,
            "wq": wq_b, "wk": wk_b, "wv": wv_b,
            "w18": w18, "w2p": w2_b,
        }
        in_maps.append(m)
    return flags, in_maps


def kernel(**inputs):
    from concourse.bass_utils import run_bass_kernel_spmd

    flags, in_maps = _prep(inputs)
    nc = _build(flags)
    res = run_bass_kernel_spmd(nc, in_maps, core_ids=list(range(N_CORES)))
    out = np.empty((B, T, C), dtype=np.float32)
    for b, r in enumerate(res.results):
        oc = np.asarray(r["out_ct"], dtype=np.float32)  # [H, DH+1, T]
        ctx = oc[:, :DH, :] / oc[:, DH:DH + 1, :]
        out[b] = ctx.transpose(2, 0, 1).reshape(T, C)
    return out
